# revision 3
# baseline (speedup 1.0000x reference)
"""Trainium2 Bass kernel for nn_Conv2d_85830626443584.

Math (from the reference):
  x: [16, 64, 128, 128] f32, W: [8, 9] f32
  s = silu(x)
  out[b, c*8+k, ho, wo] = sum_{dh,dw} W[k, 3*dh+dw] * s[b, c, ho+dh, wo+dw]
  out: [16, 512, 126, 126] f32

Strategy (per NeuronCore, batch-sharded 16/8 = 2 batches -> 128 channel-images):
  * Each channel-image is an independent [128, 128] tile, SBUF layout
    [partition=h, free=w].
  * The 3x3 conv is computed as 3 PSUM-accumulating matmuls per output map k:
    a banded stationary matrix Band[(h_in=128), (ho=126)] carries the 3
    vertical taps (dh), and the horizontal taps (dw) come for free as
    rhs access-pattern column offsets:
       psum_k[ho, n] += sum_h Band_{k,dw}[h, ho] * s[h, n+dw]   (dw = 0,1,2)
    No im2col, no data duplication: x is read from HBM once and out written
    once, which is the HBM roofline floor for this problem.
  * float32r matmul mode: full PE rate (1 col/cycle) at out free-size >= 256.
  * Images are processed in groups of 4 (rhs N = 4*126 = 504 <= 512 psum bank).
"""

import numpy as np

B, C, H, WD = 16, 64, 128, 128
NK = 8            # n_convs
HO = WO = 126     # output spatial dims
NCORES = 8
B_LOC = B // NCORES              # 2 batches per core
NIMG_LOC = B_LOC * C             # 128 images per core
GRP = 4                          # images per group
NGRP = NIMG_LOC // GRP           # 32 groups

_CACHE = {}


def _make_bands(W: np.ndarray) -> np.ndarray:
    """Banded stationary matrices, one [128, 126] per (k, dw).

    bands[h, k, dw, ho] = W[k, 3*dh + dw] where dh = h - ho in {0,1,2}.
    Returned flattened to [128, 8*3*126].
    """
    bands = np.zeros((H, NK, 3, HO), dtype=np.float32)
    ho = np.arange(HO)
    for dh in range(3):
        for dw in range(3):
            # lhs slice shape [126, 8] <- W[:, 3*dh+dw] broadcast
            bands[ho + dh, :, dw, ho] = W[:, 3 * dh + dw][None, :]
    return bands.reshape(H, NK * 3 * HO)


def _build_module():
    import concourse.bass as bass
    import concourse.mybir as mybir
    import concourse.tile as tile
    from contextlib import ExitStack

    f32 = mybir.dt.float32
    f32r = mybir.dt.float32r

    nc = bass.Bass("TRN2", target_bir_lowering=False, debug=False)

    x_d = nc.dram_tensor("x", [B_LOC, C, H, WD], f32, kind="ExternalInput")
    bands_d = nc.dram_tensor("bands", [H, NK * 3 * HO], f32, kind="ExternalInput")
    out_d = nc.dram_tensor("out", [B_LOC, NK * C, HO, WO], f32, kind="ExternalOutput")

    with tile.TileContext(nc) as tc, ExitStack() as ctx:
        cpool = ctx.enter_context(tc.tile_pool(name="const", bufs=1))
        xpool = ctx.enter_context(tc.tile_pool(name="xin", bufs=3))
        spool = ctx.enter_context(tc.tile_pool(name="silu", bufs=3))
        opool = ctx.enter_context(tc.tile_pool(name="outs", bufs=3))
        ppool = ctx.enter_context(tc.tile_pool(name="psum", bufs=8, space="PSUM"))

        band_t = cpool.tile([H, NK * 3 * HO], f32)
        nc.sync.dma_start(band_t[:], bands_d.ap())
        band4 = band_t[:].rearrange("p (k d m) -> p k d m", k=NK, d=3)

        # [128 images, 128 h, 128 w] view of the local input
        x_flat = x_d.ap().rearrange("b c h w -> (b c) h w")
        # [128 images, 8 k, 126, 126] view of the local output
        out_r = out_d.ap().rearrange("b (c k) h w -> (b c) k h w", k=NK)

        for g in range(NGRP):
            i0 = g * GRP
            xt = xpool.tile([H, GRP * WD], f32)
            nc.sync.dma_start(
                xt[:].rearrange("h (i w) -> h i w", i=GRP),
                x_flat[i0 : i0 + GRP, :, :].rearrange("i h w -> h i w"),
            )
            sg = spool.tile([H, GRP * WD], f32, tag="sg")
            nc.scalar.activation(sg[:], xt[:], mybir.ActivationFunctionType.Sigmoid)
            st = spool.tile([H, GRP * WD], f32, tag="st")
            nc.vector.tensor_mul(st[:], xt[:], sg[:])
            st3 = st[:].rearrange("h (i w) -> h i w", i=GRP)

            # ot free layout (i, k, w): lets the store DMA merge (i, k) into
            # one dim on the DRAM side (i stride = NK * k stride), keeping the
            # balanced DMA AP within 3 dims.
            ot = opool.tile([HO, GRP * NK * WO], f32)
            ot4 = ot[:].rearrange("p (i k w) -> p i k w", i=GRP, k=NK)
            for k in range(NK):
                ps = ppool.tile([HO, GRP * WO], f32)
                ps3 = ps[:].rearrange("p (i n) -> p i n", i=GRP)
                for dw in range(3):
                    nc.tensor.matmul(
                        ps3,
                        band4[:, k, dw, :].bitcast(f32r),
                        st3[:, :, dw : dw + WO].bitcast(f32r),
                        start=(dw == 0),
                        stop=(dw == 2),
                    )
                nc.vector.tensor_copy(ot4[:, :, k, :], ps3)

            nc.sync.dma_start(
                out_r[i0 : i0 + GRP, :, :, :].rearrange("i k h w -> h i k w"),
                ot4,
            )

    return nc


def _get_module():
    if "nc" not in _CACHE:
        _CACHE["nc"] = _build_module()
    return _CACHE["nc"]


def kernel(x: np.ndarray, W: np.ndarray) -> np.ndarray:
    from concourse.bass_utils import run_bass_kernel_spmd

    x = np.ascontiguousarray(np.asarray(x, dtype=np.float32))
    W = np.asarray(W, dtype=np.float32)
    assert x.shape == (B, C, H, WD), x.shape
    assert W.shape == (NK, 9), W.shape

    bands = _make_bands(W)
    nc = _get_module()

    in_maps = [
        {"x": x[i * B_LOC : (i + 1) * B_LOC], "bands": bands} for i in range(NCORES)
    ]
    res = run_bass_kernel_spmd(nc, in_maps, core_ids=list(range(NCORES)))
    out = np.concatenate([res.results[i]["out"] for i in range(NCORES)], axis=0)
    return out


# revision 18
# speedup vs baseline: 1101185144.0342x; 1101185144.0342x over previous
"""Trainium2 Bass kernel for nn_Conv2d_85830626443584.

Math (from the reference):
  x: [16, 64, 128, 128] f32, W: [8, 9] f32
  s = silu(x)
  out[b, c*8+k, ho, wo] = sum_{dh,dw} W[k, 3*dh+dw] * s[b, c, ho+dh, wo+dw]
  out: [16, 512, 126, 126] f32

Strategy (per NeuronCore, batch-sharded 16/8 = 2 batches -> 128 channel-images):
  * Each channel-image is an independent [128, 128] tile, SBUF layout
    [partition=h, free=w].
  * The 3x3 conv is computed as 3 PSUM-accumulating matmuls per output map k:
    a banded stationary matrix Band[(h_in=128), (ho=126)] carries the 3
    vertical taps (dh), and the horizontal taps (dw) come for free as
    rhs access-pattern column offsets:
       psum_k[ho, n] += sum_h Band_{k,dw}[h, ho] * s[h, n+dw]   (dw = 0,1,2)
    No im2col, no data duplication: x is read from HBM once and out written
    once, which is the HBM roofline floor for this problem.
  * float32r matmul mode: full PE rate (1 col/cycle) at out free-size >= 256.
  * Images are processed in groups of 4 (rhs N = 4*126 = 504 <= 512 psum bank).
"""

import numpy as np

B, C, H, WD = 16, 64, 128, 128
NK = 8            # n_convs
HO = WO = 126     # output spatial dims
NCORES = 8
B_LOC = B // NCORES              # 2 batches per core
NIMG_LOC = B_LOC * C             # 128 images per core
GRP = 4                          # images per group
NGRP = NIMG_LOC // GRP           # 32 groups

_CACHE = {}


def _make_bands(W: np.ndarray) -> np.ndarray:
    """Banded stationary matrices, one [128, 126] per (k, dw).

    bands[h, k, dw, ho] = W[k, 3*dh + dw] where dh = h - ho in {0,1,2}.
    Returned flattened to [128, 8*3*126].
    """
    bands = np.zeros((H, NK, 3, HO), dtype=np.float32)
    ho = np.arange(HO)
    for dh in range(3):
        for dw in range(3):
            bands[ho + dh, :, dw, ho] = W[:, 3 * dh + dw][None, :]
    return bands.reshape(H, NK * 3 * HO)


def _build_module(native_silu: bool = True):
    # native_silu=True: single ACT Silu instruction (hardware path). False:
    # Sigmoid + DVE mul, for CoreSim (which lacks a Silu implementation).
    import concourse.mybir as mybir
    import concourse.tile as tile
    from concourse import bacc
    from contextlib import ExitStack

    f32 = mybir.dt.float32
    f32r = mybir.dt.float32r

    # Bacc (not raw Bass): its compile() legalizes semaphore waits -- TRN2
    # instructions encode at most one sync wait; excess waits are split into
    # fused InstEventSemaphore instructions.
    nc = bacc.Bacc("TRN2", target_bir_lowering=False, debug=False)

    x_d = nc.dram_tensor("x", [B_LOC, C, H, WD], f32, kind="ExternalInput")
    bands_d = nc.dram_tensor("bands", [H, NK * 3 * HO], f32r, kind="ExternalInput")
    out_d = nc.dram_tensor("out", [B_LOC, NK * C, HO, WO], f32, kind="ExternalOutput")

    with tile.TileContext(nc) as tc, ExitStack() as ctx:
        cpool = ctx.enter_context(tc.tile_pool(name="const", bufs=1))
        xpool = ctx.enter_context(tc.tile_pool(name="xin", bufs=3))
        spool = ctx.enter_context(tc.tile_pool(name="silu", bufs=3))
        opool = ctx.enter_context(tc.tile_pool(name="outs", bufs=3))
        ppool = ctx.enter_context(tc.tile_pool(name="psum", bufs=8, space="PSUM"))

        band_t = cpool.tile([H, NK * 3 * HO], f32r)
        nc.sync.dma_start(band_t[:], bands_d.ap())
        band4 = band_t[:].rearrange("p (k d m) -> p k d m", k=NK, d=3)

        # [128 images, 128 h, 128 w] view of the local input
        x_flat = x_d.ap().rearrange("b c h w -> (b c) h w")
        # [128 images, 8 k, 126, 126] view of the local output
        out_r = out_d.ap().rearrange("b (c k) h w -> (b c) k h w", k=NK)

        for g in range(NGRP):
            i0 = g * GRP
            xt = xpool.tile([H, GRP * WD], f32)
            nc.sync.dma_start(
                xt[:].rearrange("h (i w) -> h i w", i=GRP),
                x_flat[i0 : i0 + GRP, :, :].rearrange("i h w -> h i w"),
            )

            st = spool.tile([H, GRP * WD], f32r, tag="st")
            if native_silu:
                nc.scalar.activation(
                    st[:], xt[:], mybir.ActivationFunctionType.Silu
                )
            else:
                sg = spool.tile([H, GRP * WD], f32, tag="sg")
                nc.scalar.activation(
                    sg[:], xt[:], mybir.ActivationFunctionType.Sigmoid
                )
                nc.vector.tensor_mul(st[:], xt[:], sg[:])
            st3 = st[:].rearrange("h (i w) -> h i w", i=GRP)

            # ot free layout (i, k, w): lets the store DMA merge (i, k) into
            # one dim on the DRAM side (i stride = NK * k stride), keeping the
            # balanced DMA AP within 3 dims.
            ot = opool.tile([HO, GRP * NK * WO], f32)
            ot4 = ot[:].rearrange("p (i k w) -> p i k w", i=GRP, k=NK)
            for k in range(NK):
                ps = ppool.tile([HO, GRP * WO], f32)
                ps3 = ps[:].rearrange("p (i n) -> p i n", i=GRP)
                for dw in range(3):
                    nc.tensor.matmul(
                        ps3,
                        band4[:, k, dw, :],
                        st3[:, :, dw : dw + WO],
                        start=(dw == 0),
                        stop=(dw == 2),
                    )
                nc.vector.tensor_copy(ot4[:, :, k, :], ps3)

            nc.sync.dma_start(
                out_r[i0 : i0 + GRP, :, :, :].rearrange("i k h w -> h i k w"),
                ot4,
            )

    nc.compile()
    return nc


def _get_module():
    if "nc" not in _CACHE:
        _CACHE["nc"] = _build_module()
    return _CACHE["nc"]


def kernel(x: np.ndarray, W: np.ndarray) -> np.ndarray:
    from concourse.bass_utils import run_bass_kernel_spmd

    x = np.ascontiguousarray(np.asarray(x, dtype=np.float32))
    W = np.asarray(W, dtype=np.float32)
    assert x.shape == (B, C, H, WD), x.shape
    assert W.shape == (NK, 9), W.shape

    bands = _make_bands(W)
    nc = _get_module()

    in_maps = [
        {"x": x[i * B_LOC : (i + 1) * B_LOC], "bands": bands} for i in range(NCORES)
    ]
    res = run_bass_kernel_spmd(nc, in_maps, core_ids=list(range(NCORES)))
    out = np.concatenate([res.results[i]["out"] for i in range(NCORES)], axis=0)
    return out
